# revision 10
# baseline (speedup 1.0000x reference)
"""Trainium2 Bass kernel for hierarchical loss.

Math: reference computes
    probs = outputs @ A.T            [B, N]
    w     = W[target]                [B, N]
    loss  = sum_b (1 - probs[b].w[b])
Since probs[b].w[b] = outputs[b] . M[target_b] with M = W @ A, and the
fixed 10-ary hierarchy gives the closed form
    M[t, l] = 0.5 + 0.25*[l//100 == t//100] + 0.125*[l//10 == t//10]
            + 0.125*[l == t]
(verified exactly against W @ A), each sample's contribution is
    win[b] = 0.5*Sall[b] + 0.25*S100[b, t//100] + 0.125*S10[b, t//10]
           + 0.125*o[b, t]
i.e. weighted hierarchical block sums of the sample's row. These are
linear functionals of o selected per-sample by the target, so the whole
loss collapses into ONE fp8 matmul per 256-sample pair-block:

  - stationary weights H[b, m] stack one-hot selectors, built ON DEVICE
    by DVE is_equal compares of the sample's (t//100, t//10, t%8)
    against an iota (m=0..9 one-hot(t//100); m=10..109 one-hot(t//10);
    m=110..117 one-hot(t%8); m=118..127 structurally zero). Only 24 KB
    of bf16 indices are shipped instead of 512 KB of one-hot bytes.
  - moving operand is the o tile itself (fp8): 1000 class columns plus
    8 columns holding the sample's 8-class segment 8*(t//8)..8*(t//8)+8
    (packed there during host input layout; device does every multiply
    and the full reduction), so the m=110..117 rows pick out o[b, t].
  - PSUM accumulates Z[m, c] over all tiles; the mask U[m, c] carries
    the level coefficients (root 0.5 rides on the t//100 rows, whose
    one-hots sum to 1 per sample: U[j, c] = 0.5 + 0.25*[c//100 == j])
    and contracts Z to the scalar loss on DVE.

fp8e4m3 + DoubleRow (256-deep contraction) makes TensorE ~3x faster
than the DMA floor, so the kernel is HBM-bound at its irreducible
traffic: the 4.13 MB/core of fp8 activations, streamed as 1 MB
mega-tile DMAs (vs 16.4 MB for the naive gather-M-rows scheme).
Device-side indirect-DMA leaf gathers were measured at ~22 us per pass
(SWDGE fixed cost x 32 instructions, and multi-offset gathers
misbehave on HW) vs +0.8% DMA for packed columns, hence the layout
above.

No M-row gather, no W @ A precompute: A/W only contribute the four
level coefficients (read from W at runtime; they are 0.5/0.25/0.125,
folded into U in fp32).
"""

import numpy as np
import ml_dtypes

NCORES = 8
B = 32768
C = 1000           # real classes
CP = 1008          # tile width: 1000 class cols + 8 leaf-segment cols
P = 128
BPC = B // NCORES  # rows per core (4096)
NMEGA = 2          # mega-tiles per core, each 16 x 128 samples (2 MB DMAs)
KPM = 16           # sample rows per partition per mega-tile (8 DoubleRow pairs)
NPAIR = KPM // 2   # DoubleRow pairs per mega-tile
MWP = 128          # H columns (118 used; DoubleRow weight step 16B-aligned)

FP8 = ml_dtypes.float8_e4m3
BF16 = ml_dtypes.bfloat16

_NC_CACHE = {}


def _build(repeats=1):
    import concourse.tile as tile
    from concourse import bacc, mybir

    fp8 = mybir.dt.float8e4
    f32 = mybir.dt.float32
    bf16 = mybir.dt.bfloat16
    nc = bacc.Bacc("TRN2", target_bir_lowering=False, debug=False,
                   num_devices=NCORES)
    o_ap = nc.dram_tensor("o", [NMEGA, P, KPM, CP], fp8,
                          kind="ExternalInput").ap()
    t_ap = nc.dram_tensor("t", [P, NMEGA * 3 * KPM], bf16,
                          kind="ExternalInput").ap()
    i_ap = nc.dram_tensor("i", [P, 100], bf16, kind="ExternalInput").ap()
    u_ap = nc.dram_tensor("u", [P, CP], f32, kind="ExternalInput").ap()
    r_ap = nc.dram_tensor("r", [P, 1], f32, kind="ExternalOutput").ap()

    N1 = 512           # first matmul's psum columns
    N2 = CP - N1       # second matmul's psum columns (496)
    DR = mybir.MatmulPerfMode.DoubleRow
    EQ = mybir.AluOpType.is_equal

    with tile.TileContext(nc) as tc:
        with tc.tile_pool(name="io", bufs=3) as io_pool, \
             tc.tile_pool(name="single", bufs=1) as single, \
             tc.tile_pool(name="work", bufs=2) as work, \
             tc.tile_pool(name="psum", bufs=2, space="PSUM") as psum_pool:
            t_sb = single.tile([P, NMEGA * 3 * KPM], bf16)
            nc.sync.dma_start(t_sb[:], t_ap[:])
            i_sb = single.tile([P, 100], bf16)
            nc.sync.dma_start(i_sb[:], i_ap[:])
            u_sb = single.tile([P, CP], f32)
            nc.sync.dma_start(u_sb[:], u_ap[:])
            racc = single.tile([P, 1], f32)

            def onehot(h_t, j, grp, m0, width):
                base = j * 3 * KPM + grp * KPM
                sel = t_sb[:, base:base + KPM]
                nc.vector.tensor_tensor(
                    out=h_t[:, :, m0:m0 + width],
                    in0=sel.unsqueeze(2).broadcast_to((P, KPM, width)),
                    in1=i_sb[:, 0:width].unsqueeze(1)
                            .broadcast_to((P, KPM, width)),
                    op=EQ)

            for _rep in range(repeats):
                p1 = psum_pool.tile([P, N1], f32, tag="p1")
                p2 = psum_pool.tile([P, N2], f32, tag="p2")
                for j in range(NMEGA):
                    o_t = io_pool.tile([P, KPM, CP], fp8, tag="o")
                    nc.sync.dma_start(o_t[:], o_ap[j])
                    h_t = io_pool.tile([P, KPM, MWP], fp8, tag="h")
                    onehot(h_t, j, 0, 0, 10)     # one-hot(t//100)
                    onehot(h_t, j, 1, 10, 100)   # one-hot(t//10)
                    onehot(h_t, j, 2, 110, 18)   # one-hot(t%8), 118:128 -> 0
                    for q in range(NPAIR):
                        first = (j == 0 and q == 0)
                        last = (j == NMEGA - 1 and q == NPAIR - 1)
                        nc.tensor.matmul(p1[:], h_t[:, 2 * q:2 * q + 2, :],
                                         o_t[:, 2 * q:2 * q + 2, 0:N1],
                                         start=first, stop=last,
                                         perf_mode=DR)
                        nc.tensor.matmul(p2[:], h_t[:, 2 * q:2 * q + 2, :],
                                         o_t[:, 2 * q:2 * q + 2, N1:CP],
                                         start=first, stop=last,
                                         perf_mode=DR)
                s1 = work.tile([P, N1], f32, tag="s1")
                nc.vector.tensor_tensor(out=s1[:], in0=p1[:],
                                        in1=u_sb[:, 0:N1],
                                        op=mybir.AluOpType.mult)
                r1 = work.tile([P, 1], f32, tag="r1")
                nc.vector.tensor_reduce(out=r1[:], in_=s1[:],
                                        axis=mybir.AxisListType.X,
                                        op=mybir.AluOpType.add)
                s2 = work.tile([P, N2], f32, tag="s2")
                nc.vector.tensor_tensor(out=s2[:], in0=p2[:],
                                        in1=u_sb[:, N1:CP],
                                        op=mybir.AluOpType.mult)
                r2 = work.tile([P, 1], f32, tag="r2")
                nc.vector.tensor_reduce(out=r2[:], in_=s2[:],
                                        axis=mybir.AxisListType.X,
                                        op=mybir.AluOpType.add)
                rsum = work.tile([P, 1], f32, tag="rs")
                nc.vector.tensor_tensor(out=rsum[:], in0=r1[:], in1=r2[:],
                                        op=mybir.AluOpType.add)
                if _rep == 0:
                    nc.vector.tensor_copy(out=racc[:], in_=rsum[:])
                else:
                    nc.vector.tensor_tensor(out=racc[:], in0=racc[:],
                                            in1=rsum[:],
                                            op=mybir.AluOpType.add)
            if repeats != 1:
                nc.vector.tensor_scalar_mul(racc[:], racc[:], 1.0 / repeats)
            nc.sync.dma_start(r_ap[:], racc[:])

    nc.compile()
    return nc


def _get_nc(repeats=1):
    if repeats not in _NC_CACHE:
        _NC_CACHE[repeats] = _build(repeats)
    return _NC_CACHE[repeats]


def _make_in_maps(outputs, target, coef):
    """Build per-core input dicts. coef = (c_root, c_mid, c_sub, c_leaf).

    Device sample layout: sample index = core*4096 + j*1024 + q*256
    + k*128 + p lives at o[core][j, p, 2*q + k, :].
    """
    c_root, c_mid, c_sub, c_leaf = (float(c) for c in coef)
    o8 = outputs.astype(FP8)                       # [B, 1000]
    t = np.asarray(target).astype(np.int32)

    # leaf segment cols: o8[s, 8*(t//8) .. +8)
    seg_idx = (8 * (t[:, None] // 8) + np.arange(8)[None, :])  # [B, 8]
    leaf = np.take_along_axis(o8, seg_idx, axis=1)             # [B, 8] fp8
    o_pack = np.concatenate([o8, leaf], axis=1)                # [B, 1008]

    # [B, CP] -> [core, j, q, k, p, CP] -> [core, j, p, 2q+k, CP]
    o_dev = o_pack.reshape(NCORES, NMEGA, NPAIR, 2, P, CP)
    o_dev = np.ascontiguousarray(o_dev.transpose(0, 1, 4, 2, 3, 5)
                                 .reshape(NCORES, NMEGA, P, KPM, CP))

    # selector indices [core, p, j, grp, kk] -> [core, P, NMEGA*3*KPM] bf16
    tkk = (t.reshape(NCORES, NMEGA, NPAIR, 2, P)
           .transpose(0, 4, 1, 2, 3).reshape(NCORES, P, NMEGA, KPM))
    tsel = np.stack([tkk // 100, tkk // 10, tkk % 8], axis=3)  # [c,P,j,3,kk]
    tsel = np.ascontiguousarray(
        tsel.reshape(NCORES, P, NMEGA * 3 * KPM).astype(BF16))

    iota = np.ascontiguousarray(
        np.broadcast_to(np.arange(100), (P, 100)).astype(BF16))

    # mask U [P, CP] carrying the level coefficients (same for all cores)
    u = np.zeros((P, CP), dtype=np.float32)
    cls = np.arange(C)
    u[0:10, :C] = c_root                 # root rides on the t//100 rows
    u[cls // 100, cls] += c_mid
    u[10 + cls // 10, cls] = c_sub
    u[110 + np.arange(8), C + np.arange(8)] = c_leaf

    return [{"o": o_dev[c], "t": tsel[c], "i": iota, "u": u}
            for c in range(NCORES)]


def kernel(outputs, target, A, W):
    outputs = np.asarray(outputs, dtype=np.float32)
    target = np.asarray(target)
    W = np.asarray(W, dtype=np.float32)
    assert outputs.shape == (B, C) and target.shape == (B,)
    # level coefficients from W (root/mid/sub on leaf 0's path, leaf self)
    coef = (W[0, 1110], W[0, 1100], W[0, 1000], W[0, 0])

    from concourse.bass_utils import run_bass_kernel_spmd
    nc = _get_nc()
    in_maps = _make_in_maps(outputs, target, coef)
    res = run_bass_kernel_spmd(nc, in_maps, list(range(NCORES)))
    total = sum(float(res.results[c]["r"].sum(dtype=np.float64))
                for c in range(NCORES))
    return np.float32(np.float64(B) - total)


# revision 11
# speedup vs baseline: 1.0310x; 1.0310x over previous
"""Trainium2 Bass kernel for hierarchical loss.

Math: reference computes
    probs = outputs @ A.T            [B, N]
    w     = W[target]                [B, N]
    loss  = sum_b (1 - probs[b].w[b])
Since probs[b].w[b] = outputs[b] . M[target_b] with M = W @ A, and the
fixed 10-ary hierarchy gives the closed form
    M[t, l] = 0.5 + 0.25*[l//100 == t//100] + 0.125*[l//10 == t//10]
            + 0.125*[l == t]
(verified exactly against W @ A), each sample's contribution is
    win[b] = 0.5*Sall[b] + 0.25*S100[b, t//100] + 0.125*S10[b, t//10]
           + 0.125*o[b, t]
i.e. weighted hierarchical block sums of the sample's row. These are
linear functionals of o selected per-sample by the target, so the whole
loss collapses into ONE fp8 matmul per 256-sample pair-block:

  - stationary weights H[b, m] stack one-hot selectors, built ON DEVICE
    by DVE is_equal compares of the sample's (t//100, t//10, t%8)
    against an iota (m=0..9 one-hot(t//100); m=10..109 one-hot(t//10);
    m=110..117 one-hot(t%8); m=118..127 structurally zero). Only 24 KB
    of bf16 indices are shipped instead of 512 KB of one-hot bytes.
  - moving operand is the o tile itself (fp8): 1000 class columns plus
    8 columns holding the sample's 8-class segment 8*(t//8)..8*(t//8)+8
    (packed there during host input layout; device does every multiply
    and the full reduction), so the m=110..117 rows pick out o[b, t].
  - PSUM accumulates Z[m, c] over all tiles; the mask U[m, c] carries
    the level coefficients (root 0.5 rides on the t//100 rows, whose
    one-hots sum to 1 per sample: U[j, c] = 0.5 + 0.25*[c//100 == j])
    and contracts Z to the scalar loss on DVE.

fp8e4m3 + DoubleRow (256-deep contraction) makes TensorE ~3x faster
than the DMA floor, so the kernel is HBM-bound at its irreducible
traffic: the 4.13 MB/core of fp8 activations, streamed as 1 MB
mega-tile DMAs (vs 16.4 MB for the naive gather-M-rows scheme).
Device-side indirect-DMA leaf gathers were measured at ~22 us per pass
(SWDGE fixed cost x 32 instructions, and multi-offset gathers
misbehave on HW) vs +0.8% DMA for packed columns, hence the layout
above.

No M-row gather, no W @ A precompute: A/W only contribute the four
level coefficients (read from W at runtime; they are 0.5/0.25/0.125,
folded into U in fp32).
"""

import numpy as np
import ml_dtypes

NCORES = 8
B = 32768
C = 1000           # real classes
CP = 1008          # tile width: 1000 class cols + 8 leaf-segment cols
P = 128
BPC = B // NCORES  # rows per core (4096)
NMEGA = 4          # mega-tiles per core, each 8 x 128 samples (1 MB DMAs)
KPM = 8            # sample rows per partition per mega-tile (4 DoubleRow pairs)
NPAIR = KPM // 2   # DoubleRow pairs per mega-tile
MWP = 128          # H columns (118 used; DoubleRow weight step 16B-aligned)

FP8 = ml_dtypes.float8_e4m3
BF16 = ml_dtypes.bfloat16

_NC_CACHE = {}


def _build(repeats=1):
    import concourse.tile as tile
    from concourse import bacc, mybir

    fp8 = mybir.dt.float8e4
    f32 = mybir.dt.float32
    bf16 = mybir.dt.bfloat16
    nc = bacc.Bacc("TRN2", target_bir_lowering=False, debug=False,
                   num_devices=NCORES)
    o_ap = nc.dram_tensor("o", [NMEGA, P, KPM, CP], fp8,
                          kind="ExternalInput").ap()
    t_ap = nc.dram_tensor("t", [P, NMEGA * 3 * KPM], bf16,
                          kind="ExternalInput").ap()
    i_ap = nc.dram_tensor("i", [P, 100], bf16, kind="ExternalInput").ap()
    u_ap = nc.dram_tensor("u", [P, CP], f32, kind="ExternalInput").ap()
    r_ap = nc.dram_tensor("r", [P, 1], f32, kind="ExternalOutput").ap()

    N1 = 512           # first matmul's psum columns
    N2 = CP - N1       # second matmul's psum columns (496)
    DR = mybir.MatmulPerfMode.DoubleRow
    EQ = mybir.AluOpType.is_equal

    with tile.TileContext(nc) as tc:
        with tc.tile_pool(name="io", bufs=4) as io_pool, \
             tc.tile_pool(name="single", bufs=1) as single, \
             tc.tile_pool(name="work", bufs=2) as work, \
             tc.tile_pool(name="psum", bufs=2, space="PSUM") as psum_pool:
            t_sb = single.tile([P, NMEGA * 3 * KPM], bf16)
            nc.sync.dma_start(t_sb[:], t_ap[:])
            i_sb = single.tile([P, 100], bf16)
            nc.sync.dma_start(i_sb[:], i_ap[:])
            u_sb = single.tile([P, CP], f32)
            nc.sync.dma_start(u_sb[:], u_ap[:])
            racc = single.tile([P, 1], f32)

            def onehot(h_t, j, grp, m0, width):
                base = j * 3 * KPM + grp * KPM
                sel = t_sb[:, base:base + KPM]
                nc.vector.tensor_tensor(
                    out=h_t[:, :, m0:m0 + width],
                    in0=sel.unsqueeze(2).broadcast_to((P, KPM, width)),
                    in1=i_sb[:, 0:width].unsqueeze(1)
                            .broadcast_to((P, KPM, width)),
                    op=EQ)

            for _rep in range(repeats):
                p1 = psum_pool.tile([P, N1], f32, tag="p1")
                p2 = psum_pool.tile([P, N2], f32, tag="p2")
                for j in range(NMEGA):
                    o_t = io_pool.tile([P, KPM, CP], fp8, tag="o")
                    nc.sync.dma_start(o_t[:], o_ap[j])
                    h_t = io_pool.tile([P, KPM, MWP], fp8, tag="h")
                    onehot(h_t, j, 0, 0, 10)     # one-hot(t//100)
                    onehot(h_t, j, 1, 10, 100)   # one-hot(t//10)
                    onehot(h_t, j, 2, 110, 18)   # one-hot(t%8), 118:128 -> 0
                    for q in range(NPAIR):
                        first = (j == 0 and q == 0)
                        last = (j == NMEGA - 1 and q == NPAIR - 1)
                        nc.tensor.matmul(p1[:], h_t[:, 2 * q:2 * q + 2, :],
                                         o_t[:, 2 * q:2 * q + 2, 0:N1],
                                         start=first, stop=last,
                                         perf_mode=DR)
                        nc.tensor.matmul(p2[:], h_t[:, 2 * q:2 * q + 2, :],
                                         o_t[:, 2 * q:2 * q + 2, N1:CP],
                                         start=first, stop=last,
                                         perf_mode=DR)
                s1 = work.tile([P, N1], f32, tag="s1")
                nc.vector.tensor_tensor(out=s1[:], in0=p1[:],
                                        in1=u_sb[:, 0:N1],
                                        op=mybir.AluOpType.mult)
                r1 = work.tile([P, 1], f32, tag="r1")
                nc.vector.tensor_reduce(out=r1[:], in_=s1[:],
                                        axis=mybir.AxisListType.X,
                                        op=mybir.AluOpType.add)
                s2 = work.tile([P, N2], f32, tag="s2")
                nc.vector.tensor_tensor(out=s2[:], in0=p2[:],
                                        in1=u_sb[:, N1:CP],
                                        op=mybir.AluOpType.mult)
                r2 = work.tile([P, 1], f32, tag="r2")
                nc.vector.tensor_reduce(out=r2[:], in_=s2[:],
                                        axis=mybir.AxisListType.X,
                                        op=mybir.AluOpType.add)
                rsum = work.tile([P, 1], f32, tag="rs")
                nc.vector.tensor_tensor(out=rsum[:], in0=r1[:], in1=r2[:],
                                        op=mybir.AluOpType.add)
                if _rep == 0:
                    nc.vector.tensor_copy(out=racc[:], in_=rsum[:])
                else:
                    nc.vector.tensor_tensor(out=racc[:], in0=racc[:],
                                            in1=rsum[:],
                                            op=mybir.AluOpType.add)
            if repeats != 1:
                nc.vector.tensor_scalar_mul(racc[:], racc[:], 1.0 / repeats)
            nc.sync.dma_start(r_ap[:], racc[:])

    nc.compile()
    return nc


def _get_nc(repeats=1):
    if repeats not in _NC_CACHE:
        _NC_CACHE[repeats] = _build(repeats)
    return _NC_CACHE[repeats]


def _make_in_maps(outputs, target, coef):
    """Build per-core input dicts. coef = (c_root, c_mid, c_sub, c_leaf).

    Device sample layout: sample index = core*4096 + j*1024 + q*256
    + k*128 + p lives at o[core][j, p, 2*q + k, :].
    """
    c_root, c_mid, c_sub, c_leaf = (float(c) for c in coef)
    o8 = outputs.astype(FP8)                       # [B, 1000]
    t = np.asarray(target).astype(np.int32)

    # leaf segment cols: o8[s, 8*(t//8) .. +8)
    seg_idx = (8 * (t[:, None] // 8) + np.arange(8)[None, :])  # [B, 8]
    leaf = np.take_along_axis(o8, seg_idx, axis=1)             # [B, 8] fp8
    o_pack = np.concatenate([o8, leaf], axis=1)                # [B, 1008]

    # [B, CP] -> [core, j, q, k, p, CP] -> [core, j, p, 2q+k, CP]
    o_dev = o_pack.reshape(NCORES, NMEGA, NPAIR, 2, P, CP)
    o_dev = np.ascontiguousarray(o_dev.transpose(0, 1, 4, 2, 3, 5)
                                 .reshape(NCORES, NMEGA, P, KPM, CP))

    # selector indices [core, p, j, grp, kk] -> [core, P, NMEGA*3*KPM] bf16
    tkk = (t.reshape(NCORES, NMEGA, NPAIR, 2, P)
           .transpose(0, 4, 1, 2, 3).reshape(NCORES, P, NMEGA, KPM))
    tsel = np.stack([tkk // 100, tkk // 10, tkk % 8], axis=3)  # [c,P,j,3,kk]
    tsel = np.ascontiguousarray(
        tsel.reshape(NCORES, P, NMEGA * 3 * KPM).astype(BF16))

    iota = np.ascontiguousarray(
        np.broadcast_to(np.arange(100), (P, 100)).astype(BF16))

    # mask U [P, CP] carrying the level coefficients (same for all cores)
    u = np.zeros((P, CP), dtype=np.float32)
    cls = np.arange(C)
    u[0:10, :C] = c_root                 # root rides on the t//100 rows
    u[cls // 100, cls] += c_mid
    u[10 + cls // 10, cls] = c_sub
    u[110 + np.arange(8), C + np.arange(8)] = c_leaf

    return [{"o": o_dev[c], "t": tsel[c], "i": iota, "u": u}
            for c in range(NCORES)]


def kernel(outputs, target, A, W):
    outputs = np.asarray(outputs, dtype=np.float32)
    target = np.asarray(target)
    W = np.asarray(W, dtype=np.float32)
    assert outputs.shape == (B, C) and target.shape == (B,)
    # level coefficients from W (root/mid/sub on leaf 0's path, leaf self)
    coef = (W[0, 1110], W[0, 1100], W[0, 1000], W[0, 0])

    from concourse.bass_utils import run_bass_kernel_spmd
    nc = _get_nc()
    in_maps = _make_in_maps(outputs, target, coef)
    res = run_bass_kernel_spmd(nc, in_maps, list(range(NCORES)))
    total = sum(float(res.results[c]["r"].sum(dtype=np.float64))
                for c in range(NCORES))
    return np.float32(np.float64(B) - total)
